# revision 4
# baseline (speedup 1.0000x reference)
"""Multi-head causal attention (B=4, S=2048, D=1024, H=16, HD=64) on 8 trn2 cores.

Sharding: tensor-parallel over heads — 2 heads per core. Each core computes
its Q/K/V projections (128 output dims), causal attention for its 2 heads,
and a partial output projection against its 128 columns of Wo. The host sums
the 8 partial outputs and adds the bias (row-parallel linear unshard).

Per-core kernel layout (all matmuls in fp32r, fp32 PSUM accumulation):
  - x is fed pre-transposed (xT: [B, D, S]) so every projection contracts
    D on the partition dim.
  - QT/KT: [128(=2 heads x 64), S] computed per batch; V via a VT projection
    + PE transposes into natural [kv, 65] tiles with a ones column appended
    (the ones column turns the P@V matmul into a fused ctx+denominator
    computation: row 64 of ctxT_ext is the softmax denominator).
  - scores are computed transposed (S_T[kv, q] = KT.T @ QT) per 128-kv-tile
    x 512-q-chunk, exp on ACT, causal handled by skipping above-diagonal
    work + one constant 128x128 mask multiply on diagonal blocks.
  - softmax normalization: reciprocal of the denominator row, broadcast
    across partitions with a K=1 ones matmul, multiply.
  - output projection: per-head K=64 matmuls accumulating in PSUM.
"""

import numpy as np

import concourse.bass as bass
import concourse.tile as tile
from concourse import bacc, mybir
from concourse.bass_utils import run_bass_kernel_spmd
from contextlib import ExitStack

F32 = mybir.dt.float32
F32R = mybir.dt.float32r
AF = mybir.ActivationFunctionType

B, S, D, H = 4, 2048, 1024, 16
HD = D // H          # 64
SCALE = float(np.sqrt(HD))
NCORES = 8
HPC = H // NCORES    # heads per core = 2
CW = HPC * HD        # per-core projection width = 128
KO = D // 128        # 8 contraction subtiles
QCH = 512            # q chunk
NQT = S // 128       # 16 q tiles / kv tiles
NCH = S // QCH       # 4 q chunks


def _emit(nc):
    XT = nc.dram_tensor("XT", [B, D, S], F32R, kind="ExternalInput").ap()
    WQT = nc.dram_tensor("WQT", [128, KO, CW], F32R, kind="ExternalInput").ap()
    WKT = nc.dram_tensor("WKT", [128, KO, CW], F32R, kind="ExternalInput").ap()
    WVT = nc.dram_tensor("WVT", [128, KO, CW], F32R, kind="ExternalInput").ap()
    WOT0 = nc.dram_tensor("WOT0", [HD, D], F32R, kind="ExternalInput").ap()
    WOT1 = nc.dram_tensor("WOT1", [HD, D], F32R, kind="ExternalInput").ap()
    CMASK = nc.dram_tensor("CMASK", [128, 128], F32R, kind="ExternalInput").ap()
    ONESB = nc.dram_tensor("ONESB", [128, HD], F32R, kind="ExternalInput").ap()
    IDENT = nc.dram_tensor("IDENT", [128, 128], F32R, kind="ExternalInput").ap()
    VONES = nc.dram_tensor("VONES", [128, NQT, HPC, 1], F32R, kind="ExternalInput").ap()
    OUT = nc.dram_tensor("OUT", [B, S, D], F32, kind="ExternalOutput").ap()

    with tile.TileContext(nc) as tc, ExitStack() as ctx, \
            nc.allow_low_precision(reason="f32r attention pipeline"):
        consts = ctx.enter_context(tc.tile_pool(name="consts", bufs=1))
        xpool = ctx.enter_context(tc.tile_pool(name="xpool", bufs=2))
        qkv = ctx.enter_context(tc.tile_pool(name="qkv", bufs=2))
        ppool = ctx.enter_context(tc.tile_pool(name="ppool", bufs=4))
        npool = ctx.enter_context(tc.tile_pool(name="npool", bufs=4))
        opool = ctx.enter_context(tc.tile_pool(name="opool", bufs=3))
        ps_a = ctx.enter_context(tc.tile_pool(name="ps_a", bufs=3, space="PSUM"))
        ps_st = ctx.enter_context(tc.tile_pool(name="ps_st", bufs=3, space="PSUM"))
        ps_cx = ctx.enter_context(tc.tile_pool(name="ps_cx", bufs=2, space="PSUM"))

        wq = consts.tile([128, KO, CW], F32R, tag="wq")
        wk = consts.tile([128, KO, CW], F32R, tag="wk")
        wv = consts.tile([128, KO, CW], F32R, tag="wv")
        wo0 = consts.tile([HD, D], F32R, tag="wo0")
        wo1 = consts.tile([HD, D], F32R, tag="wo1")
        cmask = consts.tile([128, 128], F32R, tag="cmask")
        onesb = consts.tile([128, HD], F32R, tag="onesb")
        ident = consts.tile([128, 128], F32R, tag="ident")
        nc.sync.dma_start(wq[:], WQT[:])
        nc.sync.dma_start(wk[:], WKT[:])
        nc.sync.dma_start(wv[:], WVT[:])
        nc.sync.dma_start(wo0[:], WOT0[:])
        nc.sync.dma_start(wo1[:], WOT1[:])
        nc.sync.dma_start(cmask[:], CMASK[:])
        nc.sync.dma_start(onesb[:], ONESB[:])
        nc.sync.dma_start(ident[:], IDENT[:])

        for b in range(B):
            xt_b = XT[b].rearrange("(ko p) s -> p ko s", p=128)

            # ---- projections: QT/KT/VT [128, S] ----
            qt = qkv.tile([128, S], F32R, tag="qt")
            kt = qkv.tile([128, S], F32R, tag="kt")
            vt = qkv.tile([128, S], F32R, tag="vt")
            for sc in range(NCH):
                xt = xpool.tile([128, KO, QCH], F32R, tag="xt")
                nc.sync.dma_start(xt[:], xt_b[:, :, bass.ts(sc, QCH)])
                for w, dst in ((wq, qt), (wk, kt), (wv, vt)):
                    pp = ps_a.tile([128, QCH], F32, tag="pa")
                    for ko in range(KO):
                        nc.tensor.matmul(pp[:], w[:, ko, :], xt[:, ko, :],
                                         start=(ko == 0), stop=(ko == KO - 1))
                    nc.vector.tensor_copy(dst[:, bass.ts(sc, QCH)], pp[:])

            # ---- V assembly: transpose VT into natural [kv, heads, 65] ----
            v_sb = qkv.tile([128, NQT, HPC, HD + 1], F32R, tag="v")
            nc.sync.dma_start(v_sb[:, :, :, HD:HD + 1], VONES[:])
            for i in range(NQT):
                tp = ps_a.tile([128, 128], F32R, tag="pa")
                nc.tensor.transpose(tp[:], vt[:, bass.ts(i, 128)], ident[:])
                nc.vector.tensor_copy(
                    v_sb[:, i, :, 0:HD],
                    tp[:].rearrange("p (h d) -> p h d", h=HPC))

            # ---- attention, q-chunk major ----
            for j in range(NCH):
                cxs = [ps_cx.tile([HD + 1, QCH], F32, tag="cx", name=f"cx{h}")
                       for h in range(HPC)]
                last = 4 * j + 3
                for i in range(last + 1):
                    s = 128 * (i - 4 * j) if i >= 4 * j else 0
                    for h in range(HPC):
                        hs = slice(HD * h, HD * (h + 1))
                        st = ps_st.tile([128, QCH], F32, tag="st")
                        nc.tensor.matmul(
                            st[:, s:QCH],
                            kt[hs, bass.ts(i, 128)],
                            qt[hs, j * QCH + s:(j + 1) * QCH],
                            start=True, stop=True)
                        pt = ppool.tile([128, QCH], F32R, tag="pt")
                        nc.scalar.activation(pt[:, s:QCH], st[:, s:QCH], AF.Exp)
                        if i >= 4 * j:
                            nc.vector.tensor_mul(
                                pt[:, s:s + 128], pt[:, s:s + 128], cmask[:])
                        nc.tensor.matmul(
                            cxs[h][:, s:QCH],
                            v_sb[:, i, h, :],
                            pt[:, s:QCH],
                            start=(i == 0), stop=(i == last))
                # normalize: rows 0..63 ctx, row 64 denominator
                for h in range(HPC):
                    rt = npool.tile([HD + 1, QCH], F32R, tag="rt")
                    nc.vector.reciprocal(rt[HD:HD + 1, :], cxs[h][HD:HD + 1, :])
                    rb = ps_st.tile([HD, QCH], F32, tag="st")
                    nc.tensor.matmul(rb[:], onesb[HD:HD + 1, :], rt[HD:HD + 1, :],
                                     start=True, stop=True)
                    craw = npool.tile([HD, QCH], F32, tag="craw")
                    nc.any.tensor_copy(craw[:], cxs[h][0:HD, :])
                    cn = npool.tile([HD, QCH], F32R, tag=f"cn{h}")
                    nc.vector.tensor_mul(cn[:], craw[:], rb[:])
                    cxs[h] = cn

                # ---- output projection for this chunk's 4 q-tiles ----
                for t in range(4):
                    for dc in range(2):
                        po = ps_a.tile([128, QCH], F32, tag="pa")
                        nc.tensor.matmul(po[:], cxs[0][:, bass.ts(t, 128)],
                                         wo0[:, bass.ts(dc, QCH)],
                                         start=True, stop=False)
                        nc.tensor.matmul(po[:], cxs[1][:, bass.ts(t, 128)],
                                         wo1[:, bass.ts(dc, QCH)],
                                         start=False, stop=True)
                        ob = opool.tile([128, QCH], F32, tag="ob")
                        nc.any.tensor_copy(ob[:], po[:])
                        nc.sync.dma_start(
                            OUT[b, (4 * j + t) * 128:(4 * j + t + 1) * 128,
                                bass.ts(dc, QCH)],
                            ob[:])


_CACHE = {}


def _build():
    nc = bacc.Bacc("TRN2", target_bir_lowering=False, debug=False,
                   num_devices=NCORES)
    _emit(nc)
    nc.compile()
    return nc


def _in_maps(x, Wq, Wk, Wv, Wo):
    x = np.asarray(x, dtype=np.float32)
    Wq = np.asarray(Wq, dtype=np.float32)
    Wk = np.asarray(Wk, dtype=np.float32)
    Wv = np.asarray(Wv, dtype=np.float32)
    Wo = np.asarray(Wo, dtype=np.float32)

    xT = np.ascontiguousarray(x.transpose(0, 2, 1))
    cmask = np.triu(np.ones((128, 128), np.float32))          # [kv_p, q_c]: q>=kv
    onesb = np.ones((128, HD), np.float32)
    ident = np.eye(128, dtype=np.float32)
    vones = np.ones((128, NQT, HPC, 1), np.float32)

    def wslice(W, c, scale=1.0):
        # rows c*128..c*128+128 of W, as [p, ko, m] with m the output dim
        wc = (W[c * CW:(c + 1) * CW, :] * scale).astype(np.float32)
        return np.ascontiguousarray(wc.reshape(CW, KO, 128).transpose(2, 1, 0))

    maps = []
    for c in range(NCORES):
        maps.append({
            "XT": xT,
            "WQT": wslice(Wq, c, scale=1.0 / SCALE),
            "WKT": wslice(Wk, c),
            "WVT": wslice(Wv, c),
            "WOT0": np.ascontiguousarray(Wo[:, c * CW:c * CW + HD].T),
            "WOT1": np.ascontiguousarray(Wo[:, c * CW + HD:c * CW + 2 * HD].T),
            "CMASK": cmask,
            "ONESB": onesb,
            "IDENT": ident,
            "VONES": vones,
        })
    return maps


def _run(x, Wq, Wk, Wv, Wo, bo, trace=False):
    nc = _CACHE.get("nc")
    if nc is None:
        nc = _CACHE["nc"] = _build()
    maps = _in_maps(x, Wq, Wk, Wv, Wo)
    res = run_bass_kernel_spmd(nc, maps, list(range(NCORES)), trace=trace)
    out = res.results[0]["OUT"].astype(np.float64)
    for c in range(1, NCORES):
        out += res.results[c]["OUT"]
    out += np.asarray(bo, dtype=np.float32)
    return out.astype(np.float32), res


def kernel(x, Wq, Wk, Wv, Wo, bo):
    out, _ = _run(x, Wq, Wk, Wv, Wo, bo)
    return out


# revision 9
# speedup vs baseline: 1.0014x; 1.0014x over previous
"""Multi-head causal attention (B=4, S=2048, D=1024, H=16, HD=64) on 8 trn2 cores.

Sharding: tensor-parallel over heads — 2 heads per core. Each core computes
its Q/K/V projections (128 output dims), causal attention for its 2 heads,
and a partial output projection against its 128 columns of Wo. The host sums
the 8 partial outputs and adds the bias (row-parallel linear unshard).

Per-core kernel layout (all matmuls in fp32r, fp32 PSUM accumulation):
  - x is fed pre-transposed (xT: [B, D, S]) so every projection contracts
    D on the partition dim.
  - QT/KT: [128(=2 heads x 64), S] computed per batch; V via a VT projection
    + PE transposes into natural [kv, 65] tiles with a ones column appended
    (the ones column turns the P@V matmul into a fused ctx+denominator
    computation: row 64 of ctxT_ext is the softmax denominator).
  - scores are computed transposed (S_T[kv, q] = KT.T @ QT) per 128-kv-tile
    x 512-q-chunk, exp on ACT, causal handled by skipping above-diagonal
    work + one constant 128x128 mask multiply on diagonal blocks.
  - softmax normalization: reciprocal of the denominator row, broadcast
    across partitions with a K=1 ones matmul, multiply.
  - output projection: per-head K=64 matmuls accumulating in PSUM.
"""

import numpy as np

import concourse.bass as bass
import concourse.tile as tile
from concourse import bacc, mybir
from concourse.bass_utils import run_bass_kernel_spmd
from contextlib import ExitStack

F32 = mybir.dt.float32
F32R = mybir.dt.float32r
AF = mybir.ActivationFunctionType

B, S, D, H = 4, 2048, 1024, 16
HD = D // H          # 64
SCALE = float(np.sqrt(HD))
NCORES = 8
HPC = H // NCORES    # heads per core = 2
CW = HPC * HD        # per-core projection width = 128
KO = D // 128        # 8 contraction subtiles
QCH = 512            # q chunk
NQT = S // 128       # 16 q tiles / kv tiles
NCH = S // QCH       # 4 q chunks


def _emit(nc):
    XT = nc.dram_tensor("XT", [B, D, S], F32R, kind="ExternalInput").ap()
    WQT = nc.dram_tensor("WQT", [128, KO, CW], F32R, kind="ExternalInput").ap()
    WKT = nc.dram_tensor("WKT", [128, KO, CW], F32R, kind="ExternalInput").ap()
    WVT = nc.dram_tensor("WVT", [128, KO, CW], F32R, kind="ExternalInput").ap()
    WOT0 = nc.dram_tensor("WOT0", [HD, D], F32R, kind="ExternalInput").ap()
    WOT1 = nc.dram_tensor("WOT1", [HD, D], F32R, kind="ExternalInput").ap()
    CMASK = nc.dram_tensor("CMASK", [128, 128], F32R, kind="ExternalInput").ap()
    ONESB = nc.dram_tensor("ONESB", [128, HD], F32R, kind="ExternalInput").ap()
    IDENT = nc.dram_tensor("IDENT", [128, 128], F32R, kind="ExternalInput").ap()
    VONES = nc.dram_tensor("VONES", [128, NQT, HPC, 1], F32R, kind="ExternalInput").ap()
    OUT = nc.dram_tensor("OUT", [B, S, D], F32, kind="ExternalOutput").ap()

    with tile.TileContext(nc) as tc, ExitStack() as ctx, \
            nc.allow_low_precision(reason="f32r attention pipeline"):
        consts = ctx.enter_context(tc.tile_pool(name="consts", bufs=1))
        xpool = ctx.enter_context(tc.tile_pool(name="xpool", bufs=2))
        qkv = ctx.enter_context(tc.tile_pool(name="qkv", bufs=2))
        ppool = ctx.enter_context(tc.tile_pool(name="ppool", bufs=4))
        npool = ctx.enter_context(tc.tile_pool(name="npool", bufs=4))
        opool = ctx.enter_context(tc.tile_pool(name="opool", bufs=3))
        ps_a = ctx.enter_context(tc.tile_pool(name="ps_a", bufs=2, space="PSUM"))
        ps_st = ctx.enter_context(tc.tile_pool(name="ps_st", bufs=3, space="PSUM"))
        ps_cx = ctx.enter_context(tc.tile_pool(name="ps_cx", bufs=3, space="PSUM"))

        wq = consts.tile([128, KO, CW], F32R, tag="wq")
        wk = consts.tile([128, KO, CW], F32R, tag="wk")
        wv = consts.tile([128, KO, CW], F32R, tag="wv")
        wo0 = consts.tile([HD, D], F32R, tag="wo0")
        wo1 = consts.tile([HD, D], F32R, tag="wo1")
        cmask = consts.tile([128, 128], F32R, tag="cmask")
        onesb = consts.tile([128, HD], F32R, tag="onesb")
        ident = consts.tile([128, 128], F32R, tag="ident")
        nc.sync.dma_start(wq[:], WQT[:])
        nc.sync.dma_start(wk[:], WKT[:])
        nc.sync.dma_start(wv[:], WVT[:])
        nc.sync.dma_start(wo0[:], WOT0[:])
        nc.sync.dma_start(wo1[:], WOT1[:])
        nc.sync.dma_start(cmask[:], CMASK[:])
        nc.sync.dma_start(onesb[:], ONESB[:])
        nc.sync.dma_start(ident[:], IDENT[:])

        for b in range(B):
            xt_b = XT[b].rearrange("(ko p) s -> p ko s", p=128)

            # ---- projections: QT/KT/VT [128, S] ----
            qt = qkv.tile([128, S], F32R, tag="qt")
            kt = qkv.tile([128, S], F32R, tag="kt")
            vt = qkv.tile([128, S], F32R, tag="vt")
            for sc in range(NCH):
                xt = xpool.tile([128, KO, QCH], F32R, tag="xt")
                nc.sync.dma_start(xt[:], xt_b[:, :, bass.ts(sc, QCH)])
                for w, dst in ((wq, qt), (wk, kt), (wv, vt)):
                    pp = ps_a.tile([128, QCH], F32, tag="pa")
                    for ko in range(KO):
                        nc.tensor.matmul(pp[:], w[:, ko, :], xt[:, ko, :],
                                         start=(ko == 0), stop=(ko == KO - 1))
                    nc.vector.tensor_copy(dst[:, bass.ts(sc, QCH)], pp[:])

            # ---- V assembly: transpose VT into natural [kv, heads, 65] ----
            v_sb = qkv.tile([128, NQT, HPC, HD + 1], F32R, tag="v")
            nc.sync.dma_start(v_sb[:, :, :, HD:HD + 1], VONES[:])
            for i in range(NQT):
                tp = ps_a.tile([128, 128], F32R, tag="pa")
                nc.tensor.transpose(tp[:], vt[:, bass.ts(i, 128)], ident[:])
                nc.vector.tensor_copy(
                    v_sb[:, i, :, 0:HD],
                    tp[:].rearrange("p (h d) -> p h d", h=HPC))

            # ---- attention, q-chunk major ----
            for j in range(NCH):
                cxs = [ps_cx.tile([HD + 1, QCH], F32, tag="cx", name=f"cx{h}")
                       for h in range(HPC)]
                last = 4 * j + 3
                for i in range(last + 1):
                    s = 128 * (i - 4 * j) if i >= 4 * j else 0
                    sts, pts = [], []
                    for h in range(HPC):
                        hs = slice(HD * h, HD * (h + 1))
                        st = ps_st.tile([128, QCH], F32, tag="st", name=f"st{h}")
                        nc.tensor.matmul(
                            st[:, s:QCH],
                            kt[hs, bass.ts(i, 128)],
                            qt[hs, j * QCH + s:(j + 1) * QCH],
                            start=True, stop=True)
                        sts.append(st)
                    for h in range(HPC):
                        pt = ppool.tile([128, QCH], F32R, tag="pt", name=f"pt{h}")
                        nc.scalar.activation(pt[:, s:QCH], sts[h][:, s:QCH], AF.Exp)
                        if i >= 4 * j:
                            nc.vector.tensor_mul(
                                pt[:, s:s + 128], pt[:, s:s + 128], cmask[:])
                        pts.append(pt)
                    for h in range(HPC):
                        nc.tensor.matmul(
                            cxs[h][:, s:QCH],
                            v_sb[:, i, h, :],
                            pts[h][:, s:QCH],
                            start=(i == 0), stop=(i == last))
                # normalize: rows 0..63 ctx, row 64 denominator
                for h in range(HPC):
                    craw = npool.tile([HD + 1, QCH], F32, tag="craw",
                                      name=f"craw{h}")
                    nc.any.tensor_copy(craw[:], cxs[h][:])
                    rt = npool.tile([HD + 1, QCH], F32R, tag="rt", name=f"rt{h}")
                    nc.vector.reciprocal(rt[HD:HD + 1, :], craw[HD:HD + 1, :])
                    rb = ps_st.tile([HD, QCH], F32, tag="st", name=f"rb{h}")
                    nc.tensor.matmul(rb[:], onesb[HD:HD + 1, :], rt[HD:HD + 1, :],
                                     start=True, stop=True)
                    cn = npool.tile([HD, QCH], F32R, tag=f"cn{h}")
                    nc.vector.tensor_mul(cn[:], craw[0:HD, :], rb[:])
                    cxs[h] = cn

                # ---- output projection for this chunk's 4 q-tiles ----
                for t in range(4):
                    for dc in range(2):
                        po = ps_a.tile([128, QCH], F32, tag="pa")
                        nc.tensor.matmul(po[:], cxs[0][:, bass.ts(t, 128)],
                                         wo0[:, bass.ts(dc, QCH)],
                                         start=True, stop=False)
                        nc.tensor.matmul(po[:], cxs[1][:, bass.ts(t, 128)],
                                         wo1[:, bass.ts(dc, QCH)],
                                         start=False, stop=True)
                        ob = opool.tile([128, QCH], F32, tag="ob")
                        nc.any.tensor_copy(ob[:], po[:])
                        nc.sync.dma_start(
                            OUT[b, (4 * j + t) * 128:(4 * j + t + 1) * 128,
                                bass.ts(dc, QCH)],
                            ob[:])


_CACHE = {}


def _build():
    nc = bacc.Bacc("TRN2", target_bir_lowering=False, debug=False,
                   num_devices=NCORES)
    _emit(nc)
    nc.compile()
    return nc


def _in_maps(x, Wq, Wk, Wv, Wo):
    x = np.asarray(x, dtype=np.float32)
    Wq = np.asarray(Wq, dtype=np.float32)
    Wk = np.asarray(Wk, dtype=np.float32)
    Wv = np.asarray(Wv, dtype=np.float32)
    Wo = np.asarray(Wo, dtype=np.float32)

    xT = np.ascontiguousarray(x.transpose(0, 2, 1))
    cmask = np.triu(np.ones((128, 128), np.float32))          # [kv_p, q_c]: q>=kv
    onesb = np.ones((128, HD), np.float32)
    ident = np.eye(128, dtype=np.float32)
    vones = np.ones((128, NQT, HPC, 1), np.float32)

    def wslice(W, c, scale=1.0):
        # rows c*128..c*128+128 of W, as [p, ko, m] with m the output dim
        wc = (W[c * CW:(c + 1) * CW, :] * scale).astype(np.float32)
        return np.ascontiguousarray(wc.reshape(CW, KO, 128).transpose(2, 1, 0))

    maps = []
    for c in range(NCORES):
        maps.append({
            "XT": xT,
            "WQT": wslice(Wq, c, scale=1.0 / SCALE),
            "WKT": wslice(Wk, c),
            "WVT": wslice(Wv, c),
            "WOT0": np.ascontiguousarray(Wo[:, c * CW:c * CW + HD].T),
            "WOT1": np.ascontiguousarray(Wo[:, c * CW + HD:c * CW + 2 * HD].T),
            "CMASK": cmask,
            "ONESB": onesb,
            "IDENT": ident,
            "VONES": vones,
        })
    return maps


def _run(x, Wq, Wk, Wv, Wo, bo, trace=False):
    nc = _CACHE.get("nc")
    if nc is None:
        nc = _CACHE["nc"] = _build()
    maps = _in_maps(x, Wq, Wk, Wv, Wo)
    res = run_bass_kernel_spmd(nc, maps, list(range(NCORES)), trace=trace)
    out = res.results[0]["OUT"].astype(np.float64)
    for c in range(1, NCORES):
        out += res.results[c]["OUT"]
    out += np.asarray(bo, dtype=np.float32)
    return out.astype(np.float32), res


def kernel(x, Wq, Wk, Wv, Wo, bo):
    out, _ = _run(x, Wq, Wk, Wv, Wo, bo)
    return out
